# revision 19
# baseline (speedup 1.0000x reference)
"""Causal self-attention (B=4, T=2048, C=1024, H=16, D=64) on 8 trn2 cores.

Sharding: core c = 2*b + g handles batch b and head-group g (8 heads each).
Fully communication-free: each core computes the qkv projection for its head
columns, causal attention for its 8 heads, and a partial output projection
(contraction over its 512 head-columns). The host sums the two head-group
partials per batch and adds out_b.

Device notes (per core):
  - scores are computed TRANSPOSED: sT[k, q]; softmax key-sums ride the PE
    via a ones-augmented V (lhsT = [v | 1]); attention output lands as y^T,
    which feeds the output projection lhsT directly (no transposes anywhere).
  - softmax division is deferred: unnormalized y^T and per-(head,qtile) sum
    rows are staged, one batched reciprocal runs at the end, and K=1 matmuls
    broadcast each reciprocal row across partitions for the final multiply.
  - score PSUM tiles are 2-bank [128, 1024] pairs so exp ops are big.
  - all matmul inputs bf16, accumulation fp32 in PSUM.
"""
import numpy as np
import ml_dtypes
from contextlib import ExitStack

import concourse.bass as bass
import concourse.mybir as mybir
import concourse.tile as tile
from concourse.masks import make_upper_triangular
from concourse.bass_utils import run_bass_kernel_spmd

BF16 = mybir.dt.bfloat16
F32 = mybir.dt.float32

B, T, C = 4, 2048, 1024
H, D = 16, 64
HC = H // 2          # heads per core
P = 128
NQ = 512             # q macro-tile (columns of the transposed score tile)
CK = C // P          # contraction chunks for qkv proj (8)
NT = T // P          # T tiles of 128 (16)
NQT = T // NQ        # q macro tiles (4)
HCOL = HC * D        # head columns per core (512)
NITER = HC * NQT     # attention (head, qtile) iterations (32)


def _split_waits(nc):
    """walrus in this container rejects >1 sync wait per instruction; hoist
    extras onto preceding NoOps on the same engine."""
    for func in nc.m.functions:
        for bb in func.blocks:
            newlist = []
            for inst in bb.instructions:
                si = inst.sync_info
                if si is not None and si.on_wait and len(si.on_wait) > 1:
                    extra = list(si.on_wait[:-1])
                    keep = list(si.on_wait[-1:])
                    for j, w in enumerate(extra):
                        newlist.append(mybir.InstNoOp(
                            name=f"{inst.name}-wsplit{j}",
                            sync_info=mybir.SyncInfo(on_wait=[w], on_update=[]),
                            bass_nofuse=True, engine=inst.engine))
                    si.on_wait = keep
                newlist.append(inst)
            bb.instructions = newlist


def _emit(nc, tc, ctx):
    xT_d = nc.dram_tensor("xT", [C, T], BF16, kind="ExternalInput")
    wqk_d = nc.dram_tensor("wqk", [C, 2 * HCOL], BF16, kind="ExternalInput")
    wv_d = nc.dram_tensor("wv", [C, HCOL], BF16, kind="ExternalInput")
    wout_d = nc.dram_tensor("wout", [HCOL, C], BF16, kind="ExternalInput")
    bqk_d = nc.dram_tensor("bqk", [2 * HCOL], F32, kind="ExternalInput")
    bv_d = nc.dram_tensor("bv", [1, HCOL], F32, kind="ExternalInput")
    out_d = nc.dram_tensor("out", [T, C], F32, kind="ExternalOutput")

    consts = ctx.enter_context(tc.tile_pool(name="consts", bufs=1))
    weights = ctx.enter_context(tc.tile_pool(name="weights", bufs=1))
    acts = ctx.enter_context(tc.tile_pool(name="acts", bufs=1))
    pt_pool = ctx.enter_context(tc.tile_pool(name="ptp", bufs=18))
    misc = ctx.enter_context(tc.tile_pool(name="misc", bufs=2))
    outp = ctx.enter_context(tc.tile_pool(name="outp", bufs=2))
    ps_s = ctx.enter_context(tc.tile_pool(name="ps_s", bufs=3, space="PSUM"))
    ps_av = ctx.enter_context(tc.tile_pool(name="ps_av", bufs=2, space="PSUM"))

    # constants
    tri01 = consts.tile([P, P], BF16, name="tri01")
    make_upper_triangular(nc, tri01, val=1.0, diag=True)
    ones_row = consts.tile([1, P], F32, name="ones_row")
    nc.vector.memset(ones_row, 1.0)
    bqk_sb = consts.tile([P, 2 * HCOL // P], F32, name="bqk_sb")
    nc.sync.dma_start(out=bqk_sb, in_=bqk_d.rearrange("(m p) -> p m", p=P))
    bv_row = consts.tile([1, HCOL], F32, name="bv_row")
    nc.sync.dma_start(out=bv_row, in_=bv_d[:])
    # broadcast v-bias to all 128 partitions via K=1 matmul
    bv_ps = ps_av.tile([P, NQ], F32, name="bv_ps", tag="av")
    nc.tensor.matmul(bv_ps[:, 0:HCOL], lhsT=ones_row, rhs=bv_row,
                     start=True, stop=True)
    bv_full = consts.tile([P, HCOL], F32, name="bv_full")
    nc.vector.tensor_copy(bv_full, bv_ps[:, 0:HCOL])

    # weight/activation loads
    xT_sb = weights.tile([P, CK, T], BF16, name="xT_sb")
    wqk_sb = weights.tile([P, CK, 2 * HCOL], BF16, name="wqk_sb")
    wv_sb = weights.tile([P, CK, HCOL], BF16, name="wv_sb")
    # split the big loads per contraction chunk so compute starts early
    xT_r = xT_d.rearrange("(c p) t -> p c t", p=P)
    wqk_r = wqk_d.rearrange("(c p) n -> p c n", p=P)
    wv_r = wv_d.rearrange("(c p) n -> p c n", p=P)
    for c in range(CK):
        nc.gpsimd.dma_start(out=wv_sb[:, c], in_=wv_r[:, c])
        nc.sync.dma_start(out=xT_sb[:, c], in_=xT_r[:, c])
        nc.gpsimd.dma_start(out=wqk_sb[:, c], in_=wqk_r[:, c])
    wout_sb = weights.tile([P, HCOL // P, C], BF16, name="wout_sb")
    nc.sync.dma_start(out=wout_sb, in_=wout_d.rearrange("(c p) n -> p c n", p=P))

    qkT_sb = acts.tile([P, 2 * HCOL // P, T], BF16, name="qkT_sb")
    v_sb = acts.tile([P, NT, HC, D + 1], BF16, name="v_sb")
    yT_sb = acts.tile([P, HCOL // P, T], BF16, name="yT_sb")
    nc.vector.memset(v_sb[:, :, :, D:D + 1], 1.0)

    # ---- QKV projection pieces (emitted on demand) ----
    def emit_qk_mtile(m):
        # q,k transposed: qkT[col, t]; col-tile m (q: m 0..3, k: m 4..7).
        # Two 512-wide T chunks share one 2-bank psum tile -> one wide add.
        for n2 in range(T // (2 * NQ)):
            ps = ps_s.tile([P, 2 * NQ], F32, name="ps_qk", tag="s")
            for half in range(2):
                n = 2 * n2 + half
                for c in range(CK):
                    nc.tensor.matmul(
                        ps[:, half * NQ:(half + 1) * NQ],
                        lhsT=wqk_sb[:, c, m * P:(m + 1) * P],
                        rhs=xT_sb[:, c, n * NQ:(n + 1) * NQ],
                        start=(c == 0), stop=(c == CK - 1))
            nc.any.tensor_tensor(
                qkT_sb[:, m, 2 * n2 * NQ:2 * (n2 + 1) * NQ], ps,
                bqk_sb[:, m:m + 1].to_broadcast((P, 2 * NQ)),
                mybir.AluOpType.add)

    def emit_v(t2):
        # v natural: v[t, col]; two row-tiles share one psum tile
        ps = ps_s.tile([P, 2 * HCOL], F32, name="ps_v", tag="s")
        for half in range(2):
            t = 2 * t2 + half
            for c in range(CK):
                nc.tensor.matmul(
                    ps[:, half * HCOL:(half + 1) * HCOL],
                    lhsT=xT_sb[:, c, t * P:(t + 1) * P],
                    rhs=wv_sb[:, c, :],
                    start=(c == 0), stop=(c == CK - 1))
        nc.any.tensor_tensor(
            v_sb[:, 2 * t2:2 * t2 + 2, :, 0:D],
            ps.rearrange("p (tt h d) -> p tt h d", tt=2, h=HC),
            bv_full.rearrange("p (h d) -> p h d", h=HC)[:, None, :, :]
            .to_broadcast((P, 2, HC, D)),
            mybir.AluOpType.add)

    def emit_scores(h, qt):
        """scores + exp for one (head, q-macro) iteration; returns pt list."""
        po = 64 * (h % 2)
        qT_h = qkT_sb[po:po + D, h // 2, :]
        kT_h = qkT_sb[po:po + D, 4 + h // 2, :]
        diag0 = (qt * NQ) // P      # first diagonal key chunk
        nkc = diag0 + NQ // P       # key chunks needed (causal)
        pts = []   # (pt_tile, half, qoff) per key chunk
        for kc2 in range((nkc + 1) // 2):
            kcs = [kc for kc in (2 * kc2, 2 * kc2 + 1) if kc < nkc]
            ps = ps_s.tile([P, 2 * NQ], F32, name="ps_sc", tag="s")
            pt = pt_pool.tile([P, 2 * NQ], BF16, name="pt", tag="pt")
            qoffs = []
            for half, kc in enumerate(kcs):
                r = kc - diag0
                qoff = max(0, r * P)
                qoffs.append(qoff)
                nc.tensor.matmul(
                    ps[:, half * NQ + qoff:(half + 1) * NQ],
                    lhsT=kT_h[:, kc * P:(kc + 1) * P],
                    rhs=qT_h[:, qt * NQ + qoff:(qt + 1) * NQ],
                    start=True, stop=True)
                pts.append((pt, half, qoff))
            if len(kcs) == 2 and qoffs[0] == 0 and qoffs[1] == 0:
                nc.scalar.activation(
                    pt, ps, mybir.ActivationFunctionType.Exp,
                    scale=float(D) ** -0.5)
            else:
                for half, kc in enumerate(kcs):
                    qoff = qoffs[half]
                    nc.scalar.activation(
                        pt[:, half * NQ + qoff:(half + 1) * NQ],
                        ps[:, half * NQ + qoff:(half + 1) * NQ],
                        mybir.ActivationFunctionType.Exp,
                        scale=float(D) ** -0.5)
            for half, kc in enumerate(kcs):
                r = kc - diag0
                if r >= 0:
                    qoff = qoffs[half]
                    nc.any.tensor_tensor(
                        pt[:, half * NQ + qoff:half * NQ + qoff + P],
                        pt[:, half * NQ + qoff:half * NQ + qoff + P],
                        tri01, mybir.AluOpType.mult)
                    if qoff > 0:
                        nc.any.memset(
                            pt[:, half * NQ:half * NQ + qoff], 0.0)
        return pts

    def emit_avs(h, qt, pts, sums_p):
        po = 64 * (h % 2)
        it = (h % 2) * NQT + qt
        nkc = len(pts)
        psum_av = ps_av.tile([P, NQ], F32, name="psum_av", tag="av")
        for kc, (pt, half, _) in enumerate(pts):
            nc.tensor.matmul(
                psum_av[0:D + 1, :],
                lhsT=v_sb[:, kc, h, :],
                rhs=pt[:, half * NQ:(half + 1) * NQ],
                start=(kc == 0), stop=(kc == nkc - 1))
        # stage unnormalized y^T and the sums row; divide later
        nc.any.tensor_copy(
            yT_sb[po:po + D, h // 2, qt * NQ:(qt + 1) * NQ],
            psum_av[0:D, :])
        srow = misc.tile([1, NQ], F32, name="srow", tag="srow")
        nc.any.tensor_copy(srow, psum_av[D:D + 1, :])
        nc.sync.dma_start(out=sums_p[it:it + 1, :], in_=srow)

    def emit_normalize_pair(hp, sums_p):
        # batched reciprocal over this head-pair's 8 sums rows, then
        # broadcast each row across partitions (K=1 matmul) and multiply.
        recip_p = misc.tile([2 * NQT, NQ], F32, name="recip_p", tag="recipp")
        nc.vector.reciprocal(recip_p, sums_p)
        for h in (2 * hp, 2 * hp + 1):
            po = 64 * (h % 2)
            for qt in range(NQT):
                it = (h % 2) * NQT + qt
                rrow = misc.tile([1, NQ], F32, name="rrow", tag="rrow")
                nc.sync.dma_start(out=rrow, in_=recip_p[it:it + 1, :])
                bps = ps_av.tile([D, NQ], F32, name="bps", tag="av")
                nc.tensor.matmul(bps, lhsT=ones_row[:, 0:D], rhs=rrow,
                                 start=True, stop=True)
                ysl = yT_sb[po:po + D, h // 2, qt * NQ:(qt + 1) * NQ]
                nc.any.tensor_tensor(ysl, ysl, bps, mybir.AluOpType.mult)

    # ---- schedule: start attention as soon as its inputs exist ----
    emit_qk_mtile(0)   # q cols for heads 0,1
    emit_qk_mtile(4)   # k cols for heads 0,1
    for t2 in range(NT // 2):
        emit_v(t2)
    # Software pipeline: the PE stream is in-order, so an av group stalls
    # everything behind it until its exp lands. Trail avs TWO iterations
    # behind scores (exp latency ~ 2 iterations of PE work) and carry the
    # pipeline across pair boundaries; a pair is normalized only after its
    # last avs flush, inside the next pair's stream.
    TRAIL = 2
    sums_tiles = {}
    pending = []
    done_avs = {}   # hp -> count of flushed av iterations

    def flush_one():
        ph, pqt, ppts = pending.pop(0)
        php = ph // 2
        emit_avs(ph, pqt, ppts, sums_tiles[php])
        done_avs[php] = done_avs.get(php, 0) + 1
        if done_avs[php] == 2 * NQT:
            emit_normalize_pair(php, sums_tiles[php])

    for hp in range(HC // 2):          # head pairs (0,1) (2,3) (4,5) (6,7)
        sums_tiles[hp] = misc.tile([2 * NQT, NQ], F32, name="sums_p",
                                   tag="sums")
        for h in (2 * hp, 2 * hp + 1):
            for qt in range(NQT):
                pts = emit_scores(h, qt)
                pending.append((h, qt, pts))
                while len(pending) > TRAIL:
                    flush_one()
                # weave next pair's qkv tiles into the ACT-bound phase
                if h == 2 * hp and hp + 1 < HC // 2:
                    if qt == 1:
                        emit_qk_mtile(hp + 1)
                    elif qt == 3:
                        emit_qk_mtile(4 + hp + 1)
    while pending:
        flush_one()

    # ---- output projection (partial: this core's 512 head-cols) ----
    for t in range(NT):
        ot = outp.tile([P, C], F32, name="ot", tag="ot")
        ps = ps_s.tile([P, C], F32, name="ps_op", tag="s")
        for half in range(C // NQ):
            for c in range(HCOL // P):
                nc.tensor.matmul(
                    ps[:, half * NQ:(half + 1) * NQ],
                    lhsT=yT_sb[:, c, t * P:(t + 1) * P],
                    rhs=wout_sb[:, c, half * NQ:(half + 1) * NQ],
                    start=(c == 0), stop=(c == HCOL // P - 1))
        nc.any.tensor_copy(ot, ps)
        nc.sync.dma_start(out=out_d[t * P:(t + 1) * P, :], in_=ot)


_NC = None


def _build():
    global _NC
    if _NC is None:
        nc = bass.Bass("TRN2")
        with tile.TileContext(nc) as tc, ExitStack() as ctx:
            _emit(nc, tc, ctx)
        _split_waits(nc)
        _NC = nc
    return _NC


def _in_maps(x, qkv_w, qkv_b, out_w):
    x = np.asarray(x, np.float32)
    qkv_w = np.asarray(qkv_w, np.float32)
    qkv_b = np.asarray(qkv_b, np.float32)
    out_w = np.asarray(out_w, np.float32)
    maps = []
    xTs = [np.ascontiguousarray(x[b].T).astype(ml_dtypes.bfloat16)
           for b in range(B)]
    for core in range(2 * B):
        b, g = core // 2, core % 2
        lo = g * HCOL
        wq = qkv_w[:, lo:lo + HCOL]
        wk = qkv_w[:, C + lo:C + lo + HCOL]
        wv = qkv_w[:, 2 * C + lo:2 * C + lo + HCOL]
        bq = qkv_b[lo:lo + HCOL]
        bk = qkv_b[C + lo:C + lo + HCOL]
        bv = qkv_b[2 * C + lo:2 * C + lo + HCOL]
        wout = out_w[lo:lo + HCOL, :]
        maps.append({
            "xT": xTs[b],
            "wqk": np.concatenate([wq, wk], 1).astype(ml_dtypes.bfloat16),
            "wv": wv.astype(ml_dtypes.bfloat16),
            "wout": np.ascontiguousarray(wout).astype(ml_dtypes.bfloat16),
            "bqk": np.concatenate([bq, bk]).astype(np.float32),
            "bv": bv[None, :].astype(np.float32),
        })
    return maps


def run(x, qkv_w, qkv_b, out_w, out_b, trace=False, tmpdir=None):
    nc = _build()
    maps = _in_maps(x, qkv_w, qkv_b, out_w)
    res = run_bass_kernel_spmd(nc, maps, core_ids=list(range(2 * B)),
                               trace=trace, tmpdir=tmpdir)
    out_b = np.asarray(out_b, np.float32)
    out = np.empty((B, T, C), np.float32)
    for b in range(B):
        out[b] = res.results[2 * b]["out"] + res.results[2 * b + 1]["out"] \
            + out_b[None, :]
    return out, res


def kernel(x, qkv_w, qkv_b, out_w, out_b):
    out, _ = run(x, qkv_w, qkv_b, out_w, out_b, trace=False)
    return out
